# revision 3
# baseline (speedup 1.0000x reference)
"""GATv2 layer (broadcast-score variant) as a Bass/Tile kernel on 8 NeuronCores.

Math: since scores[i,j] = e[j] (row-broadcast) masked by A, the masked softmax +
aggregation collapse to
    g = exp(e - ln2),  e = relu(X @ W.T) @ a_w          (the ln2 bias cancels)
    out = relu( (A @ (g*Wh)) / (A @ g) )                with Wh = X @ W.T
Each core computes a 1024-row block of the output.

v3 (vs v2 baseline at ~64.7us):
- A and X are repacked on the HOST into [128, *] partition-major layouts so
  every dma_start reads per-partition-contiguous bytes -> 128 big descriptors
  per trigger instead of 512+ small ones.  v2's DMA_DIRECT2D triggers took
  ~1us each and serialized on the Sync engine (~25us), starving the DMA queue
  (67% duty).
- The whole 8MB A.T block stays resident in SBUF (no at-pool recycling, no
  inter-trigger semaphore waits); all at triggers issue up-front on the sync
  HWDGE ring while xt/wt/aw/out ride the scalar HWDGE ring.
- Phase-1 (Wh -> e -> g -> G) and phase-2 (nm/dn accumulation) are emitted
  INTERLEAVED so the PE queue never drains while the DVE/ACT e-chain catches
  up: iter b = [ph1 batch b | nm groups of batch b-1 | dn groups of batch b-2].
- Gg memset split so the first dn chunk's zeros are ready early.
"""

import os

import numpy as np

import concourse.tile as tile
from concourse import bacc, mybir
from concourse.bass_utils import run_bass_kernel_spmd

N, IN_DIM, OUT_DIM = 8192, 256, 128
NCORES = 8
RPC = N // NCORES          # rows per core (1024)
P = 128                    # partitions
NJ = N // P                # 64 contraction chunks
DH = IN_DIM // P           # 2 chunks of the d-contraction
HF = RPC // 2              # 512-wide i-halves for phase-2 streams
LN2 = 0.6931471805599453

B1 = 4                     # phase-1 j-tile batch (chunks per batch)
NB = NJ // B1              # 16 batches
NSL_A = 16                 # at slices (4 chunks = 0.5MB each)
ACH = NJ // NSL_A          # chunks per at slice (4)
NSL_X = 4                  # xt slices (2048 nodes each)
XCH = NJ // NSL_X          # j-chunks per xt slice (16)

F32 = mybir.dt.float32
BF16 = mybir.dt.bfloat16
FP8 = mybir.dt.float8e4
AFT = mybir.ActivationFunctionType

NM_LAG = 1                 # nm group batch-lag behind phase 1
DN_LAG = 2                 # dn group batch-lag (Gg memset must land first)


def emit_body(nc, tc, io, pools):
    at, xt, wt, awb, out = io
    big, ph1, outp = pools

    # Gg memset first on the (slow-to-launch) GpSimd queue; split so the
    # leading chunks are zeroed well before the first dn LDWEIGHTS.
    Gg = big.tile([P, NJ, OUT_DIM], FP8, tag="Gg", name="Gg")
    nc.gpsimd.memset(Gg[:, 0:8, :], 0.0)
    nc.gpsimd.memset(Gg[:, 8:24, :], 0.0)
    nc.gpsimd.memset(Gg[:, 24:NJ, :], 0.0)

    # small tensors + xt on the scalar HWDGE ring (phase 1 needs them first)
    wt_sb = big.tile([P, DH, OUT_DIM], BF16, tag="wt_sb", name="wt_sb")
    nc.scalar.dma_start(out=wt_sb, in_=wt.rearrange("(dh p) o -> p dh o", p=P))
    aw_sb = big.tile([P, OUT_DIM], BF16, tag="aw_sb", name="aw_sb")
    nc.scalar.dma_start(out=aw_sb, in_=awb[:, :])
    xt_s = []
    for s in range(NSL_X):
        xs = big.tile([P, DH, N // NSL_X], FP8, tag=f"xt{s}", name=f"xt{s}")
        nc.scalar.dma_start(
            out=xs, in_=xt[:, s * DH * (N // NSL_X):(s + 1) * DH * (N // NSL_X)]
            .rearrange("p (dh n) -> p dh n", dh=DH))
        xt_s.append(xs)
    # whole A.T column-block -> SBUF, 16 x 0.5MB triggers on the sync ring
    # (per-partition contiguous -> 128 x 4KB descriptors per trigger)
    at_s = []
    for s in range(NSL_A):
        a4 = big.tile([P, ACH, RPC], FP8, tag=f"at{s}", name=f"at{s}")
        nc.sync.dma_start(
            out=a4, in_=at[:, s * ACH * RPC:(s + 1) * ACH * RPC]
            .rearrange("p (c i) -> p c i", c=ACH))
        at_s.append(a4)

    G = big.tile([P, NJ, OUT_DIM], FP8, tag="G", name="G")
    g64 = big.tile([P, NJ], F32, tag="g64", name="g64")
    ones_bf = big.tile([1, P], BF16, tag="ones", name="ones")
    nc.vector.memset(ones_bf, 1.0)
    nln2 = big.tile([P, 1], F32, tag="nln2", name="nln2")
    nc.vector.memset(nln2, -LN2)
    rc = big.tile([1, RPC], F32, tag="rc", name="rc")

    with tc.tile_pool(name="ps", bufs=1, space="PSUM") as ps:
        nm = [ps.tile([P, HF], F32, tag=f"nm{h}", name=f"nm{h}", bufs=1)
              for h in range(2)]
        dn = [ps.tile([P, HF], F32, tag=f"dn{h}", name=f"dn{h}", bufs=1)
              for h in range(2)]

        def ph1_batch(b):
            wh4 = ps.tile([P, B1, OUT_DIM], F32, tag="wh4", name="wh4", bufs=3)
            for k in range(B1):
                t = b * B1 + k
                s, off = t // XCH, (t % XCH) * P
                for dh in range(DH):
                    nc.tensor.matmul(
                        wh4[:, k, :],
                        xt_s[s][:, dh, off:off + P],
                        wt_sb[:, dh, :],
                        start=(dh == 0),
                        stop=(dh == DH - 1),
                    )
            t0 = b * B1
            scr = ph1.tile([P, B1, OUT_DIM], BF16, name="scr")
            e4 = ph1.tile([P, B1], F32, name="e4")
            for k in range(B1):
                nc.vector.scalar_tensor_tensor(
                    out=scr[:, k, :], in0=wh4[:, k, :], scalar=0.0,
                    in1=aw_sb,
                    op0=mybir.AluOpType.max, op1=mybir.AluOpType.mult,
                    accum_out=e4[:, k:k + 1],
                )
            nc.scalar.activation(g64[:, t0:t0 + B1], e4, AFT.Exp, bias=nln2[:, 0:1])
            for k in range(B1):
                t = t0 + k
                # 3-of-4 G-copies on Scalar, 1-of-4 on DVE (DVE owns the e-STTs)
                if t % 4 == 3:
                    nc.vector.tensor_scalar_mul(
                        G[:, t, :], wh4[:, k, :], g64[:, t:t + 1]
                    )
                else:
                    nc.scalar.activation(
                        G[:, t, :], wh4[:, k, :], AFT.Copy,
                        scale=g64[:, t:t + 1],
                    )
            nc.gpsimd.tensor_copy(
                out=Gg[:, t0:t0 + B1, 0:1], in_=g64[:, t0:t0 + B1]
            )

        def nm_group(cp):
            s, r = cp // 2, 2 * (cp % 2)
            for h in range(2):
                nc.tensor.matmul(
                    nm[h][:, :],
                    G[:, 2 * cp:2 * cp + 2, :],
                    at_s[s][:, r:r + 2, h * HF:(h + 1) * HF],
                    start=(cp == 0),
                    stop=(cp == NJ // 2 - 1),
                    perf_mode=mybir.MatmulPerfMode.DoubleRow,
                )

        def dn_group(cp):
            s, r = cp // 2, 2 * (cp % 2)
            for h in range(2):
                nc.tensor.matmul(
                    dn[h][:, :],
                    Gg[:, 2 * cp:2 * cp + 2, :],
                    at_s[s][:, r:r + 2, h * HF:(h + 1) * HF],
                    start=(cp == 0),
                    stop=(cp == NJ // 2 - 1),
                    perf_mode=mybir.MatmulPerfMode.DoubleRow,
                )

        for b in range(NB):
            ph1_batch(b)
            if b >= NM_LAG:
                nm_group(2 * (b - NM_LAG))
                nm_group(2 * (b - NM_LAG) + 1)
            if b >= DN_LAG:
                dn_group(2 * (b - DN_LAG))
                dn_group(2 * (b - DN_LAG) + 1)
        # drain: dn first so the recip->broadcast->mul chain starts earlier
        for cp in range(2 * (NB - DN_LAG), NJ // 2):
            dn_group(cp)
        for cp in range(2 * (NB - NM_LAG), NJ // 2):
            nm_group(cp)

        # ---- output: out = relu(nm) * (1/dn) broadcast over o ----
        for h in range(2):
            nc.vector.reciprocal_approx_fast(
                out=rc[0:1, h * HF:(h + 1) * HF], in_=dn[h][0:1, :]
            )
            rel = outp.tile([P, HF], F32, tag="rel", name="rel")
            nc.scalar.activation(rel, nm[h], AFT.Relu)
            rcb = outp.tile([1, HF], BF16, tag="rcb", name="rcb")
            nc.vector.tensor_copy(out=rcb, in_=rc[0:1, h * HF:(h + 1) * HF])
            rbc = ps.tile([P, HF], F32, tag="rbc", name="rbc", bufs=1)
            nc.tensor.matmul(
                rbc, ones_bf[0:1, 0:P], rcb[0:1, :], start=True, stop=True,
            )
            o_sb = outp.tile([P, HF], BF16, tag="osb", name="osb")
            nc.vector.tensor_mul(o_sb, rel, rbc)
            nc.sync.dma_start(out=out[:, h * HF:(h + 1) * HF], in_=o_sb)


def build_nc(repeat=1):
    nc = bacc.Bacc("TRN2", target_bir_lowering=False)
    # at[p, c*RPC + i] = A[core*RPC + i, c*128 + p]  (partition-major repack)
    at = nc.dram_tensor("at", [P, NJ * RPC], FP8, kind="ExternalInput")
    # xt[p, ((s*DH)+dh)*2048 + n'] = X[s*2048 + n', dh*128 + p]
    xt = nc.dram_tensor("xt", [P, DH * N], FP8, kind="ExternalInput")
    wt = nc.dram_tensor("wt", [IN_DIM, OUT_DIM], BF16, kind="ExternalInput")  # W.T
    awb = nc.dram_tensor("awb", [P, OUT_DIM], BF16, kind="ExternalInput")
    out = nc.dram_tensor("out", [OUT_DIM, RPC], BF16, kind="ExternalOutput")  # transposed

    with tile.TileContext(nc) as tc:
        with (
            tc.tile_pool(name="big", bufs=1) as big,
            tc.tile_pool(name="ph1", bufs=4) as ph1,
            tc.tile_pool(name="outp", bufs=2) as outp,
        ):
            for _ in range(repeat):
                emit_body(nc, tc, (at, xt, wt, awb, out), (big, ph1, outp))
    nc.compile()
    return nc


_NC_CACHE = None


def _get_nc():
    global _NC_CACHE
    if _NC_CACHE is None:
        _NC_CACHE = build_nc()
    return _NC_CACHE


def make_in_maps(X, A, W, a_w):
    X = np.ascontiguousarray(np.asarray(X, dtype=np.float32))
    A = np.ascontiguousarray(np.asarray(A, dtype=np.float32))
    W = np.ascontiguousarray(np.asarray(W, dtype=np.float32))
    a_w = np.ascontiguousarray(np.asarray(a_w, dtype=np.float32))

    bf = mybir.dt.np(BF16)
    f8 = mybir.dt.np(FP8)
    # xt: [s, n'] x [dh, p] -> [p, s, dh, n']
    NSX = N // NSL_X
    xtp = (X.T.astype(f8)                       # [256, 8192]
           .reshape(DH, P, NSL_X, NSX)          # [dh, p, s, n']
           .transpose(1, 2, 0, 3)               # [p, s, dh, n']
           .reshape(P, DH * N))
    xtp = np.ascontiguousarray(xtp)
    wtp = np.ascontiguousarray(W.T.astype(bf))  # [256, 128]
    awp = np.ascontiguousarray(
        np.broadcast_to(a_w[None, :], (P, OUT_DIM)).astype(bf))

    A8 = A.astype(f8)
    in_maps = []
    for c in range(NCORES):
        blk = A8[c * RPC:(c + 1) * RPC, :]      # [i=1024, j=8192]
        atp = (blk.reshape(RPC, NJ, P)           # [i, c, p]
               .transpose(2, 1, 0)               # [p, c, i]
               .reshape(P, NJ * RPC))
        in_maps.append({
            "at": np.ascontiguousarray(atp),
            "xt": xtp, "wt": wtp, "awb": awp,
        })
    return in_maps


def kernel_with_results(X, A, W, a_w, trace=False):
    in_maps = make_in_maps(X, A, W, a_w)
    res = run_bass_kernel_spmd(_get_nc(), in_maps, list(range(NCORES)), trace=trace)
    out = np.concatenate(
        [np.ascontiguousarray(r["out"].T) for r in res.results], axis=0
    )
    return out.astype(np.float32), res


def kernel(X, A, W, a_w):
    out, _ = kernel_with_results(X, A, W, a_w)
    return out


# revision 7
# speedup vs baseline: 1.0267x; 1.0267x over previous
"""GATv2 layer (broadcast-score variant) as a Bass/Tile kernel on 8 NeuronCores.

Math: since scores[i,j] = e[j] (row-broadcast) masked by A, the masked softmax +
aggregation collapse to
    g = exp(e - ln2),  e = relu(X @ W.T) @ a_w          (the ln2 bias cancels)
    out = relu( (A @ (g*Wh)) / (A @ g) )                with Wh = X @ W.T
Each core computes a 1024-row block of the output.

v3 (vs v2 baseline at ~64.7us):
- A and X are repacked on the HOST into [128, *] partition-major layouts so
  every dma_start reads per-partition-contiguous bytes -> 128 big descriptors
  per trigger instead of 512+ small ones.  v2's DMA_DIRECT2D triggers took
  ~1us each and serialized on the Sync engine (~25us), starving the DMA queue
  (67% duty).
- The whole 8MB A.T block stays resident in SBUF (no at-pool recycling, no
  inter-trigger semaphore waits); all at triggers issue up-front on the sync
  HWDGE ring while xt/wt/aw/out ride the scalar HWDGE ring.
- Phase-1 (Wh -> e -> g -> G) and phase-2 (nm/dn accumulation) are emitted
  INTERLEAVED so the PE queue never drains while the DVE/ACT e-chain catches
  up: iter b = [ph1 batch b | nm groups of batch b-1 | dn groups of batch b-2].
- Gg memset split so the first dn chunk's zeros are ready early.
"""

import os

import numpy as np

import concourse.tile as tile
from concourse import bacc, mybir
from concourse.bass_utils import run_bass_kernel_spmd

N, IN_DIM, OUT_DIM = 8192, 256, 128
NCORES = 8
RPC = N // NCORES          # rows per core (1024)
P = 128                    # partitions
NJ = N // P                # 64 contraction chunks
DH = IN_DIM // P           # 2 chunks of the d-contraction
HF = RPC // 2              # 512-wide i-halves for phase-2 streams
LN2 = 0.6931471805599453

B1 = 4                     # phase-1 j-tile batch (chunks per batch)
NB = NJ // B1              # 16 batches
# at slices in j-chunks: two small leading slices so phase 2 starts early,
# then 1MB slices (128 descriptors of 8KB each -> cheap triggers)
A_SLICES = [4, 4, 8, 8, 8, 8, 8, 8, 8]
NSL_X = 4                  # xt slices (2048 nodes each)
XCH = NJ // NSL_X          # j-chunks per xt slice (16)

F32 = mybir.dt.float32
BF16 = mybir.dt.bfloat16
FP8 = mybir.dt.float8e4
AFT = mybir.ActivationFunctionType

NM_LAG = 1                 # nm group batch-lag behind phase 1
DN_LAG = 2                 # dn group batch-lag (Gg memset must land first)


def emit_body(nc, tc, io, pools):
    at, xt, wt, awb, out = io
    big, ph1, outp = pools

    # Gg memset first on the (slow-to-launch) GpSimd queue; split so the
    # leading chunks are zeroed well before the first dn LDWEIGHTS.
    Gg = big.tile([P, NJ, OUT_DIM], FP8, tag="Gg", name="Gg")
    nc.gpsimd.memset(Gg[:, 0:8, :], 0.0)
    nc.gpsimd.memset(Gg[:, 8:24, :], 0.0)
    nc.gpsimd.memset(Gg[:, 24:NJ, :], 0.0)

    # wt + first xt piece ride the SYNC ring FIRST (its preamble retires
    # earliest); the first matmul needs exactly wt + xt chunk 0.
    wt_sb = big.tile([P, DH, OUT_DIM], BF16, tag="wt_sb", name="wt_sb")
    nc.sync.dma_start(out=wt_sb, in_=wt.rearrange("(dh p) o -> p dh o", p=P))
    XSN = N // NSL_X
    xt_s = [big.tile([P, DH, XSN], FP8, tag=f"xt{s}", name=f"xt{s}")
            for s in range(NSL_X)]
    xt0_r = xt[:, 0:DH * XSN].rearrange("p (dh n) -> p dh n", dh=DH)
    nc.sync.dma_start(out=xt_s[0][:, :, 0:512], in_=xt0_r[:, :, 0:512])
    nc.sync.dma_start(out=xt_s[0][:, :, 512:XSN], in_=xt0_r[:, :, 512:XSN])
    # whole A.T column-block -> SBUF on the sync ring behind the xt head
    # (per-partition contiguous -> 128 big descriptors per trigger)
    at_s = []
    at_chunk0 = []
    pos = 0
    for s, ach in enumerate(A_SLICES):
        a4 = big.tile([P, ach, RPC], FP8, tag=f"at{s}", name=f"at{s}")
        nc.sync.dma_start(
            out=a4, in_=at[:, pos * RPC:(pos + ach) * RPC]
            .rearrange("p (c i) -> p c i", c=ach))
        at_s.append(a4)
        at_chunk0.append(pos)
        pos += ach

    def at_pair(cp):
        """[P, 2, RPC] slice holding j-chunks (2cp, 2cp+1)."""
        c0 = 2 * cp
        s = 0
        while at_chunk0[s] + A_SLICES[s] <= c0:
            s += 1
        r = c0 - at_chunk0[s]
        return at_s[s][:, r:r + 2, :]

    # the rest of xt + aw on the scalar HWDGE ring
    aw_sb = big.tile([P, OUT_DIM], BF16, tag="aw_sb", name="aw_sb")
    nc.scalar.dma_start(out=aw_sb, in_=awb[:, :])
    for s in range(1, NSL_X):
        nc.scalar.dma_start(
            out=xt_s[s], in_=xt[:, s * DH * XSN:(s + 1) * DH * XSN]
            .rearrange("p (dh n) -> p dh n", dh=DH))

    G = big.tile([P, NJ, OUT_DIM], FP8, tag="G", name="G")
    g64 = big.tile([P, NJ], F32, tag="g64", name="g64")
    ones_bf = big.tile([1, P], BF16, tag="ones", name="ones")
    nc.vector.memset(ones_bf, 1.0)
    nln2 = big.tile([P, 1], F32, tag="nln2", name="nln2")
    nc.vector.memset(nln2, -LN2)
    rc = big.tile([1, RPC], F32, tag="rc", name="rc")

    with tc.tile_pool(name="ps", bufs=1, space="PSUM") as ps:
        nm = [ps.tile([P, HF], F32, tag=f"nm{h}", name=f"nm{h}", bufs=1)
              for h in range(2)]
        dn = [ps.tile([P, HF], F32, tag=f"dn{h}", name=f"dn{h}", bufs=1)
              for h in range(2)]

        def ph1_batch(b):
            wh4 = ps.tile([P, B1, OUT_DIM], F32, tag="wh4", name="wh4", bufs=3)
            for k in range(B1):
                t = b * B1 + k
                s, off = t // XCH, (t % XCH) * P
                for dh in range(DH):
                    nc.tensor.matmul(
                        wh4[:, k, :],
                        xt_s[s][:, dh, off:off + P],
                        wt_sb[:, dh, :],
                        start=(dh == 0),
                        stop=(dh == DH - 1),
                    )
            t0 = b * B1
            scr = ph1.tile([P, B1, OUT_DIM], FP8, name="scr")
            e4 = ph1.tile([P, B1], F32, name="e4")
            for k in range(B1):
                nc.vector.scalar_tensor_tensor(
                    out=scr[:, k, :], in0=wh4[:, k, :], scalar=0.0,
                    in1=aw_sb,
                    op0=mybir.AluOpType.max, op1=mybir.AluOpType.mult,
                    accum_out=e4[:, k:k + 1],
                )
            nc.scalar.activation(g64[:, t0:t0 + B1], e4, AFT.Exp, bias=nln2[:, 0:1])
            for k in range(B1):
                t = t0 + k
                # 3-of-4 G-copies on Scalar, 1-of-4 on DVE (DVE owns the e-STTs)
                if t % 4 == 3:
                    nc.vector.tensor_scalar_mul(
                        G[:, t, :], wh4[:, k, :], g64[:, t:t + 1]
                    )
                else:
                    nc.scalar.activation(
                        G[:, t, :], wh4[:, k, :], AFT.Copy,
                        scale=g64[:, t:t + 1],
                    )
            nc.gpsimd.tensor_copy(
                out=Gg[:, t0:t0 + B1, 0:1], in_=g64[:, t0:t0 + B1]
            )

        def nm_group(cp):
            a2 = at_pair(cp)
            for h in range(2):
                nc.tensor.matmul(
                    nm[h][:, :],
                    G[:, 2 * cp:2 * cp + 2, :],
                    a2[:, :, h * HF:(h + 1) * HF],
                    start=(cp == 0),
                    stop=(cp == NJ // 2 - 1),
                    perf_mode=mybir.MatmulPerfMode.DoubleRow,
                )

        def dn_group(cp):
            a2 = at_pair(cp)
            for h in range(2):
                nc.tensor.matmul(
                    dn[h][:, :],
                    Gg[:, 2 * cp:2 * cp + 2, :],
                    a2[:, :, h * HF:(h + 1) * HF],
                    start=(cp == 0),
                    stop=(cp == NJ // 2 - 1),
                    perf_mode=mybir.MatmulPerfMode.DoubleRow,
                )

        for b in range(NB):
            ph1_batch(b)
            if b >= NM_LAG:
                nm_group(2 * (b - NM_LAG))
                nm_group(2 * (b - NM_LAG) + 1)
            if b >= DN_LAG:
                dn_group(2 * (b - DN_LAG))
                dn_group(2 * (b - DN_LAG) + 1)
        # drain: dn first so the recip->broadcast->mul chain starts earlier
        for cp in range(2 * (NB - DN_LAG), NJ // 2):
            dn_group(cp)
        for cp in range(2 * (NB - NM_LAG), NJ // 2):
            nm_group(cp)

        # ---- output: out = relu(nm) * (1/dn) broadcast over o ----
        for h in range(2):
            nc.vector.reciprocal_approx_fast(
                out=rc[0:1, h * HF:(h + 1) * HF], in_=dn[h][0:1, :]
            )
            rel = outp.tile([P, HF], F32, tag="rel", name="rel")
            nc.scalar.activation(rel, nm[h], AFT.Relu)
            rcb = outp.tile([1, HF], BF16, tag="rcb", name="rcb")
            nc.vector.tensor_copy(out=rcb, in_=rc[0:1, h * HF:(h + 1) * HF])
            rbc = ps.tile([P, HF], F32, tag="rbc", name="rbc", bufs=1)
            nc.tensor.matmul(
                rbc, ones_bf[0:1, 0:P], rcb[0:1, :], start=True, stop=True,
            )
            o_sb = outp.tile([P, HF], BF16, tag="osb", name="osb")
            nc.vector.tensor_mul(o_sb, rel, rbc)
            nc.sync.dma_start(out=out[:, h * HF:(h + 1) * HF], in_=o_sb)


def build_nc(repeat=1):
    nc = bacc.Bacc("TRN2", target_bir_lowering=False)
    # at[p, c*RPC + i] = A[core*RPC + i, c*128 + p]  (partition-major repack)
    at = nc.dram_tensor("at", [P, NJ * RPC], FP8, kind="ExternalInput")
    # xt[p, ((s*DH)+dh)*2048 + n'] = X[s*2048 + n', dh*128 + p]
    xt = nc.dram_tensor("xt", [P, DH * N], FP8, kind="ExternalInput")
    wt = nc.dram_tensor("wt", [IN_DIM, OUT_DIM], BF16, kind="ExternalInput")  # W.T
    awb = nc.dram_tensor("awb", [P, OUT_DIM], BF16, kind="ExternalInput")
    out = nc.dram_tensor("out", [OUT_DIM, RPC], BF16, kind="ExternalOutput")  # transposed

    with tile.TileContext(nc) as tc:
        with (
            tc.tile_pool(name="big", bufs=1) as big,
            tc.tile_pool(name="ph1", bufs=4) as ph1,
            tc.tile_pool(name="outp", bufs=2) as outp,
        ):
            for _ in range(repeat):
                emit_body(nc, tc, (at, xt, wt, awb, out), (big, ph1, outp))
    nc.compile()
    return nc


_NC_CACHE = None


def _get_nc():
    global _NC_CACHE
    if _NC_CACHE is None:
        _NC_CACHE = build_nc()
    return _NC_CACHE


def make_in_maps(X, A, W, a_w):
    X = np.ascontiguousarray(np.asarray(X, dtype=np.float32))
    A = np.ascontiguousarray(np.asarray(A, dtype=np.float32))
    W = np.ascontiguousarray(np.asarray(W, dtype=np.float32))
    a_w = np.ascontiguousarray(np.asarray(a_w, dtype=np.float32))

    bf = mybir.dt.np(BF16)
    f8 = mybir.dt.np(FP8)
    # xt: [s, n'] x [dh, p] -> [p, s, dh, n']
    NSX = N // NSL_X
    xtp = (X.T.astype(f8)                       # [256, 8192]
           .reshape(DH, P, NSL_X, NSX)          # [dh, p, s, n']
           .transpose(1, 2, 0, 3)               # [p, s, dh, n']
           .reshape(P, DH * N))
    xtp = np.ascontiguousarray(xtp)
    wtp = np.ascontiguousarray(W.T.astype(bf))  # [256, 128]
    awp = np.ascontiguousarray(
        np.broadcast_to(a_w[None, :], (P, OUT_DIM)).astype(bf))

    A8 = A.astype(f8)
    in_maps = []
    for c in range(NCORES):
        blk = A8[c * RPC:(c + 1) * RPC, :]      # [i=1024, j=8192]
        atp = (blk.reshape(RPC, NJ, P)           # [i, c, p]
               .transpose(2, 1, 0)               # [p, c, i]
               .reshape(P, NJ * RPC))
        in_maps.append({
            "at": np.ascontiguousarray(atp),
            "xt": xtp, "wt": wtp, "awb": awp,
        })
    return in_maps


def kernel_with_results(X, A, W, a_w, trace=False):
    in_maps = make_in_maps(X, A, W, a_w)
    res = run_bass_kernel_spmd(_get_nc(), in_maps, list(range(NCORES)), trace=trace)
    out = np.concatenate(
        [np.ascontiguousarray(r["out"].T) for r in res.results], axis=0
    )
    return out.astype(np.float32), res


def kernel(X, A, W, a_w):
    out, _ = kernel_with_results(X, A, W, a_w)
    return out
